# revision 14
# baseline (speedup 1.0000x reference)
"""Trainium2 Bass kernel for a batched LSTM (nn_ChaoticLSTM).

Reference computation (per core, data-parallel over batch):
    xproj = x @ Wi + B                      # [b, T, 4H]
    per t: gates = xproj[:, t] + h @ Wh
           i, f, g, o = sig/sig/tanh/sig splits
           c = f*c + i*g ; h = o*tanh(c)
    outputs: (hs [b, T, H], (h_T, c_T))

Device mapping (everything transposed: gates/hidden on partitions,
batch on the free dim, so the recurrence needs no per-step transposes):
  - 8 cores, 8 batch rows each.
  - gates^T PSUM tile [128, 8, TBLK*8]: 8 chunks of 128 gate rows x
    (time, batch) columns.  Bulk "xproj" matmuls (f32r, 1 cyc/row) and
    k=1 bias matmuls pre-fill the accumulator; 16 tiny bf16 matmuls per
    step (Wh stationary, FWL) add the recurrent term.
  - ACT does tanh/sigmoid straight out of PSUM; DVE does the cell update.
  - h is stored bf16 into a history tile that doubles as the next step's
    matmul moving operand; blocks of TBLK steps are PE-transposed back to
    batch-major layout and DMAed out.
Gate blocks are host-permuted to (g, i, f, o) so one sigmoid op covers
i,f,o contiguously.
"""

import sys

if "/opt/trn_rl_repo" not in sys.path:
    sys.path.insert(0, "/opt/trn_rl_repo")

import numpy as np
import ml_dtypes

B_FULL = 64
T_FULL = 512
F_IN = 128
H = 256
G4 = 4 * H  # 1024
NCORES = 8
B_PER = B_FULL // NCORES  # 8

_NC_CACHE = {}


def _build_nc(T, TBLK):
    import concourse.mybir as mybir
    import concourse.tile as tile
    from concourse import bacc
    from concourse.masks import make_identity

    assert T % TBLK == 0
    NBLK = T // TBLK
    BT = TBLK * B_PER  # columns per psum block (time-major, batch-minor)
    f32 = mybir.dt.float32
    f32r = mybir.dt.float32r
    bf16 = mybir.dt.bfloat16
    SIG = mybir.ActivationFunctionType.Sigmoid
    TANH = mybir.ActivationFunctionType.Tanh

    nc = bacc.Bacc()

    x_d = nc.dram_tensor("x", [B_PER, T, F_IN], f32, kind="ExternalInput")
    wi_d = nc.dram_tensor("wi", [F_IN, G4], f32r, kind="ExternalInput")
    wh_d = nc.dram_tensor("wh", [H, G4], bf16, kind="ExternalInput")
    bhi_d = nc.dram_tensor("bhi", [1, G4], bf16, kind="ExternalInput")
    blo_d = nc.dram_tensor("blo", [1, G4], bf16, kind="ExternalInput")
    hs_d = nc.dram_tensor("out_hs", [B_PER, T, H], f32, kind="ExternalOutput")
    ct_d = nc.dram_tensor("out_ct", [B_PER, H], f32, kind="ExternalOutput")

    with tile.TileContext(nc) as tc:
        with tc.tile_pool(name="singles", bufs=1) as singles:
            # --- persistent sbuf ---
            wh_sb = []
            for c in range(2):
                w = singles.tile([128, G4], bf16, tag=f"wh{c}")
                nc.sync.dma_start(out=w, in_=wh_d[c * 128 : (c + 1) * 128, :])
                wh_sb.append(w)
            wi_sb = singles.tile([128, G4], f32r)
            nc.sync.dma_start(out=wi_sb, in_=wi_d[:, :])
            bhi_sb = singles.tile([1, G4], bf16)
            nc.sync.dma_start(out=bhi_sb, in_=bhi_d[:, :])
            blo_sb = singles.tile([1, G4], bf16)
            nc.sync.dma_start(out=blo_sb, in_=blo_d[:, :])
            ones_sb = singles.tile([1, BT], bf16)
            nc.vector.memset(ones_sb, 1.0)
            ident = singles.tile([128, 128], f32)
            make_identity(nc, ident)
            ident_bf = singles.tile([128, 128], bf16)
            make_identity(nc, ident_bf)
            xT = singles.tile([128, T * B_PER], f32r)  # x transposed: [feat, (t,b)]
            c_sb = singles.tile([128, 2, B_PER], f32)  # cell state (hidden-chunk, b)

            # --- prologue: load x and transpose to feature-major ---
            NXT = (T * B_PER) // 128  # number of 128-row tiles
            t_rows = 128 // B_PER  # timesteps per x tile (16)
            with (
                tc.tile_pool(name="xload", bufs=3) as xload,
                tc.tile_pool(name="trp", bufs=2, space="PSUM") as trp,
            ):
                for i in range(NXT):
                    xt = xload.tile([128, F_IN], f32)
                    src = x_d[:, i * t_rows : (i + 1) * t_rows, :].rearrange(
                        "b t f -> t b f"
                    )
                    nc.sync.dma_start(out=xt, in_=src)
                    ps = trp.tile([128, 128], f32)
                    nc.tensor.transpose(ps, xt, ident)
                    nc.vector.tensor_copy(out=xT[:, i * 128 : (i + 1) * 128], in_=ps)

            with (
                tc.tile_pool(name="gates", bufs=2, space="PSUM") as gates_pool,
                tc.tile_pool(name="hist", bufs=2) as hist_pool,
                tc.tile_pool(name="steps", bufs=3) as steps,
                tc.tile_pool(name="outp", bufs=2) as outp,
            ):
                def emit_bulk(P, b):
                    """xproj + bias matmuls pre-filling psum tile P for block b.

                    Returns a list of thunks so they can be interleaved
                    between steps (they only touch PE)."""
                    thunks = []
                    for j in range(8):

                        def f(j=j):
                            # start=True only on the first matmul touching
                            # each 2KB psum bank (j even): start marks the
                            # whole bank pending-zero, so the j-odd first
                            # write lands as overwrite via that same mark.
                            nc.tensor.matmul(
                                P[:, j, :],
                                wi_sb[:, j * 128 : (j + 1) * 128],
                                xT[:, b * BT : (b + 1) * BT],
                                start=(j % 2 == 0),
                                stop=False,
                                skip_group_check=True,
                            )

                        thunks.append(f)
                    for bias_sb in (bhi_sb, blo_sb):

                        def f2(bias_sb=bias_sb):
                            for j in range(8):
                                nc.tensor.matmul(
                                    P[:, j, :],
                                    bias_sb[:, j * 128 : (j + 1) * 128],
                                    ones_sb,
                                    start=False,
                                    stop=False,
                                    skip_group_check=True,
                                )

                        thunks.append(f2)
                    return thunks

                P_blocks = []
                hist_blocks = []
                histf_blocks = []
                for b in range(NBLK):
                    P_blocks.append(gates_pool.tile([128, 8, BT], f32, tag="gates", name=f"gates{b}"))
                    hist_blocks.append(
                        hist_pool.tile(
                            [128, TBLK, 2, B_PER], bf16, tag="hist", name=f"hist{b}"
                        )
                    )
                    histf_blocks.append(
                        hist_pool.tile(
                            [128, TBLK, 2, B_PER], f32, tag="histf", name=f"histf{b}"
                        )
                    )

                pending = emit_bulk(P_blocks[0], 0)
                for th in pending:
                    th()
                pending = []

                for b in range(NBLK):
                    P = P_blocks[b]
                    hist = hist_blocks[b]
                    if b + 1 < NBLK:
                        pending = emit_bulk(P_blocks[b + 1], b + 1)

                    for tl in range(TBLK):
                        tg = b * TBLK + tl  # global step
                        col = slice(tl * B_PER, (tl + 1) * B_PER)
                        if tg > 0:
                            if tl == 0:
                                h_prev = hist_blocks[b - 1][:, TBLK - 1]
                            else:
                                h_prev = hist[:, tl - 1]
                            # g chunks (j=0,1) first so tanh can start early
                            for j in range(8):
                                for c in range(2):
                                    # stop only on the very last matmul per
                                    # psum bank per block (see start note)
                                    nc.tensor.matmul(
                                        P[:, j, col],
                                        wh_sb[c][:, j * 128 : (j + 1) * 128],
                                        h_prev[:, c, :],
                                        start=False,
                                        stop=(
                                            c == 1 and j % 2 == 1 and tl == TBLK - 1
                                        ),
                                        skip_group_check=True,
                                    )
                                if j == 1:
                                    g_t = steps.tile([128, 2, B_PER], f32, tag="g")
                                    nc.scalar.activation(g_t, P[:, 0:2, col], TANH)
                            # slip one bulk matmul for the next block into
                            # the PE stream while it waits on h
                            if pending:
                                pending.pop(0)()
                        else:
                            g_t = steps.tile([128, 2, B_PER], f32, tag="g")
                            nc.scalar.activation(g_t, P[:, 0:2, col], TANH)

                        ifo_t = steps.tile([128, 6, B_PER], f32, tag="ifo")
                        nc.scalar.activation(ifo_t, P[:, 2:8, col], SIG)

                        m1 = steps.tile([128, 2, B_PER], f32, tag="m1")
                        nc.vector.tensor_mul(m1, ifo_t[:, 0:2], g_t)
                        if tg == 0:
                            nc.vector.tensor_copy(out=c_sb, in_=m1)
                        else:
                            m2 = steps.tile([128, 2, B_PER], f32, tag="m2")
                            nc.vector.tensor_mul(m2, ifo_t[:, 2:4], c_sb)
                            nc.vector.tensor_add(c_sb, m1, m2)
                        th_t = steps.tile([128, 2, B_PER], f32, tag="th")
                        nc.scalar.activation(th_t, c_sb, TANH)
                        # bf16 copy feeds the next matmul (critical path);
                        # f32 copy is the output-precision path (off-chain)
                        nc.vector.tensor_mul(hist[:, tl], ifo_t[:, 4:6], th_t)
                        nc.vector.tensor_mul(
                            histf_blocks[b][:, tl], ifo_t[:, 4:6], th_t
                        )

                    for th in pending:
                        th()
                    pending = []

                    # block output: transpose hist back to batch-major
                    QT = TBLK * 2 * B_PER // 128  # quarters (4 for TBLK=32)
                    tq = 128 // (2 * B_PER)  # timesteps per quarter (8)
                    ob = outp.tile([128, QT, 128], f32, tag="ob")
                    for q in range(QT):
                        pt = P[:, q, 0:128]  # [128,128] f32 scratch
                        nc.tensor.transpose(
                            pt,
                            histf_blocks[b][:, q * tq : (q + 1) * tq].rearrange(
                                "p t c b -> p (t c b)"
                            ),
                            ident,
                        )
                        nc.vector.tensor_copy(out=ob[:, q, :], in_=pt)
                        dst = hs_d[
                            :, b * TBLK + q * tq : b * TBLK + (q + 1) * tq, :
                        ].rearrange("b t (c p) -> t c b p", c=2)
                        nc.sync.dma_start(out=dst, in_=ob[:, q, :])

                # final cell state: transpose [128,(2,8)] -> [(2,8),128]
                ctp = gates_pool.tile([128, 8, BT], f32, tag="gates")
                pt = ctp[0:16, 0, 0:128]
                nc.tensor.transpose(pt, c_sb.rearrange("p c b -> p (c b)"), ident)
                cto = outp.tile([16, 128], f32, tag="cto")
                nc.vector.tensor_copy(out=cto, in_=pt)
                nc.sync.dma_start(
                    out=ct_d.rearrange("b (c p) -> c b p", c=2), in_=cto
                )

    nc.finalize()
    return nc


def _get_nc(T=T_FULL, TBLK=32):
    key = (T, TBLK)
    if key not in _NC_CACHE:
        _NC_CACHE[key] = _build_nc(T, TBLK)
    return _NC_CACHE[key]


def _host_prep(Wi, Wh, B):
    """Permute gate blocks (i,f,g,o) -> (g,i,f,o); cast for device."""
    perm = np.concatenate(
        [
            np.arange(2 * H, 3 * H),  # g
            np.arange(0, H),  # i
            np.arange(H, 2 * H),  # f
            np.arange(3 * H, 4 * H),  # o
        ]
    )
    Wi_p = np.ascontiguousarray(np.asarray(Wi, np.float32)[:, perm])
    Wh_p = np.asarray(Wh, np.float32)[:, perm]
    B_p = np.asarray(B, np.float32)[perm]
    Wh_bf = np.ascontiguousarray(Wh_p.astype(ml_dtypes.bfloat16))
    b_hi = B_p.astype(ml_dtypes.bfloat16)
    b_lo = (B_p - b_hi.astype(np.float32)).astype(ml_dtypes.bfloat16)
    return Wi_p, Wh_bf, b_hi.reshape(1, G4), b_lo.reshape(1, G4)


def kernel(x, Wi, Wh, B):
    from concourse.bass_utils import run_bass_kernel_spmd

    x = np.asarray(x, np.float32)
    Wi_p, Wh_bf, b_hi, b_lo = _host_prep(Wi, Wh, B)
    nc = _get_nc()

    in_maps = []
    for c in range(NCORES):
        in_maps.append(
            {
                "x": np.ascontiguousarray(x[c * B_PER : (c + 1) * B_PER]),
                "wi": Wi_p,
                "wh": Wh_bf,
                "bhi": b_hi,
                "blo": b_lo,
            }
        )
    res = run_bass_kernel_spmd(nc, in_maps, list(range(NCORES)))
    hs = np.concatenate([res.results[c]["out_hs"] for c in range(NCORES)], axis=0)
    ct = np.concatenate([res.results[c]["out_ct"] for c in range(NCORES)], axis=0)
    ht = np.ascontiguousarray(hs[:, -1, :])
    return (hs, (ht, ct))


# revision 27
# speedup vs baseline: 90.0921x; 90.0921x over previous
"""Trainium2 Bass kernel for a batched LSTM (nn_ChaoticLSTM).

Reference computation (per core, data-parallel over batch):
    xproj = x @ Wi + B                      # [b, T, 4H]
    per t: gates = xproj[:, t] + h @ Wh
           i, f, g, o = sig/sig/tanh/sig splits
           c = f*c + i*g ; h = o*tanh(c)
    outputs: (hs [b, T, H], (h_T, c_T))

Device mapping (everything transposed: gates/hidden on partitions,
batch on the free dim, so the recurrence needs no per-step transposes):
  - 8 cores, 8 batch rows each.
  - gates^T PSUM tile [128, 8, TBLK*8]: 8 chunks of 128 gate rows x
    (time, batch) columns.  Bulk "xproj" matmuls (f32r, 1 cyc/row) and
    k=1 bias matmuls pre-fill the accumulator; 16 tiny bf16 matmuls per
    step (Wh stationary, FWL) add the recurrent term.
  - sigma-everywhere: tanh(x) = 2*sig(2x)-1 with the scale factors folded
    into the host-prepared weights, so ONE sigmoid ACT op covers all four
    gates; h is stored as h/2 (Wh doubled to compensate) straight from a
    fused (sig-0.5)*o DVE op.
  - h/2 is stored bf16 into a history tile that doubles as the next step's
    matmul moving operand, and f32 into a parallel tile for output
    precision; blocks of TBLK steps are PE-transposed back to batch-major
    layout (the x2 restore rides the psum->sbuf copy) and DMAed out.
Gate blocks are host-permuted to (g, i, f, o).
"""

import sys

if "/opt/trn_rl_repo" not in sys.path:
    sys.path.insert(0, "/opt/trn_rl_repo")

import numpy as np
import ml_dtypes

B_FULL = 64
T_FULL = 512
F_IN = 128
H = 256
G4 = 4 * H  # 1024
NCORES = 8
B_PER = B_FULL // NCORES  # 8

_NC_CACHE = {}


def _build_nc(T, TBLK, repeats=1, internal_out=False):
    import concourse.mybir as mybir
    import concourse.tile as tile
    from concourse import bacc
    from concourse.masks import make_identity

    assert T % TBLK == 0
    NBLK = T // TBLK
    BT = TBLK * B_PER  # columns per psum block (time-major, batch-minor)
    f32 = mybir.dt.float32
    f32r = mybir.dt.float32r
    bf16 = mybir.dt.bfloat16
    SIG = mybir.ActivationFunctionType.Sigmoid
    TANH = mybir.ActivationFunctionType.Tanh

    nc = bacc.Bacc()

    x_d = nc.dram_tensor("x", [B_PER, T, F_IN], f32, kind="ExternalInput")
    wi_d = nc.dram_tensor("wi", [F_IN, G4], f32r, kind="ExternalInput")
    wh_d = nc.dram_tensor("wh", [H, G4], bf16, kind="ExternalInput")
    bhi_d = nc.dram_tensor("bhi", [1, G4], bf16, kind="ExternalInput")
    blo_d = nc.dram_tensor("blo", [1, G4], bf16, kind="ExternalInput")
    hs_kind = "Internal" if internal_out else "ExternalOutput"
    hs_d = nc.dram_tensor("out_hs", [B_PER, T, H], f32, kind=hs_kind)
    ct_d = nc.dram_tensor("out_ct", [B_PER, H], f32, kind="ExternalOutput")

    with tile.TileContext(nc) as tc:
        with tc.tile_pool(name="singles", bufs=1) as singles:
            # --- persistent sbuf ---
            wh_sb = []
            for c in range(2):
                w = singles.tile([128, G4], bf16, tag=f"wh{c}")
                nc.sync.dma_start(out=w, in_=wh_d[c * 128 : (c + 1) * 128, :])
                wh_sb.append(w)
            wi_sb = singles.tile([128, G4], f32r)
            nc.sync.dma_start(out=wi_sb, in_=wi_d[:, :])
            bhi_sb = singles.tile([1, G4], bf16)
            nc.sync.dma_start(out=bhi_sb, in_=bhi_d[:, :])
            blo_sb = singles.tile([1, G4], bf16)
            nc.sync.dma_start(out=blo_sb, in_=blo_d[:, :])
            ones_sb = singles.tile([1, BT], bf16)
            nc.vector.memset(ones_sb, 1.0)
            ident = singles.tile([128, 128], f32)
            make_identity(nc, ident)
            # x transposed [feat, (t,b)], one tile per TBLK-block so each
            # block's xproj matmuls only wait on their own two transposes
            xT_blocks = []
            for xb in range(NBLK):
                xT_blocks.append(
                    singles.tile([128, BT], f32r, tag=f"xT{xb}", name=f"xT{xb}")
                )
            c_sb = singles.tile([128, 2, B_PER], f32)  # cell state (hidden-chunk, b)

            # --- prologue: load x and transpose to feature-major ---
            NXT = (T * B_PER) // 128  # number of 128-row tiles
            t_rows = 128 // B_PER  # timesteps per x tile (16)
            with (
                tc.tile_pool(name="xload", bufs=3) as xload,
                tc.tile_pool(name="trp", bufs=2, space="PSUM") as trp,
            ):
                for i in range(NXT):
                    xt = xload.tile([128, F_IN], f32)
                    src = x_d[:, i * t_rows : (i + 1) * t_rows, :].rearrange(
                        "b t f -> t b f"
                    )
                    nc.sync.dma_start(out=xt, in_=src)
                    ps = trp.tile([128, 128], f32)
                    nc.tensor.transpose(ps, xt, ident)
                    xb, xo = divmod(i * 128, BT)
                    nc.vector.tensor_copy(
                        out=xT_blocks[xb][:, xo : xo + 128], in_=ps
                    )

            with (
                tc.tile_pool(name="gates", bufs=2, space="PSUM") as gates_pool,
                tc.tile_pool(name="hist", bufs=2) as hist_pool,
                tc.tile_pool(name="steps", bufs=3) as steps,
                tc.tile_pool(name="outp", bufs=2) as outp,
            ):
                def emit_bulk(P, b):
                    """xproj + bias matmuls pre-filling psum tile P for block b.

                    Returns a list of thunks so they can be interleaved
                    between steps (they only touch PE)."""
                    thunks = []
                    for j in range(8):

                        def f(j=j):
                            # start=True only on the first matmul touching
                            # each 2KB psum bank (j even): start marks the
                            # whole bank pending-zero, so the j-odd first
                            # write lands as overwrite via that same mark.
                            nc.tensor.matmul(
                                P[:, j, :],
                                wi_sb[:, j * 128 : (j + 1) * 128],
                                xT_blocks[b],
                                start=(j % 2 == 0),
                                stop=False,
                                skip_group_check=True,
                            )

                        thunks.append(f)
                    for bias_sb in (bhi_sb, blo_sb):
                        for j in range(8):

                            def f2(bias_sb=bias_sb, j=j):
                                nc.tensor.matmul(
                                    P[:, j, :],
                                    bias_sb[:, j * 128 : (j + 1) * 128],
                                    ones_sb,
                                    start=False,
                                    stop=False,
                                    skip_group_check=True,
                                )

                            thunks.append(f2)
                    return thunks

                P_blocks = []
                hist_blocks = []
                histf_blocks = []
                for r in range(repeats):
                    for b in range(NBLK):
                        P_blocks.append(gates_pool.tile([128, 8, BT], f32, tag="gates", name=f"gates{r}_{b}"))
                        hist_blocks.append(
                            hist_pool.tile(
                                [128, TBLK, 2, B_PER], bf16, tag="hist", name=f"hist{r}_{b}"
                            )
                        )
                        histf_blocks.append(
                            hist_pool.tile(
                                [128, TBLK, 2, B_PER], f32, tag="histf", name=f"histf{r}_{b}"
                            )
                        )

                pending = emit_bulk(P_blocks[0], 0)
                for th in pending:
                    th()
                pending = []

                for gb in range(repeats * NBLK):
                    r, b = divmod(gb, NBLK)
                    P = P_blocks[gb]
                    hist = hist_blocks[gb]
                    if b + 1 < NBLK:
                        pending = emit_bulk(P_blocks[gb + 1], b + 1)
                    elif gb + 1 < repeats * NBLK:
                        pending = emit_bulk(P_blocks[gb + 1], 0)

                    for tl in range(TBLK):
                        tg = gb * TBLK + tl  # global step (across repeats)
                        col = slice(tl * B_PER, (tl + 1) * B_PER)
                        if tg > 0:
                            if tl == 0:
                                h_prev = hist_blocks[gb - 1][:, TBLK - 1]
                            else:
                                h_prev = hist[:, tl - 1]
                            for j in range(8):
                                for c in range(2):
                                    # stop only on the very last matmul per
                                    # psum bank per block (see start note)
                                    nc.tensor.matmul(
                                        P[:, j, col],
                                        wh_sb[c][:, j * 128 : (j + 1) * 128],
                                        h_prev[:, c, :],
                                        start=False,
                                        stop=(
                                            c == 1 and j % 2 == 1 and tl == TBLK - 1
                                        ),
                                        skip_group_check=True,
                                    )
                            # slip one bulk matmul for the next block into
                            # the PE stream while it waits on h
                            if pending:
                                pending.pop(0)()

                        # Everything is a sigmoid: host pre-scales the g-gate
                        # weights by 2 so g = 2*sig-1 (= tanh), and stores
                        # h/2 (Wh doubled to compensate).  One ACT op covers
                        # all four gates.
                        sg = steps.tile([128, 8, B_PER], f32, tag="sg")
                        nc.scalar.activation(sg, P[:, :, col], SIG)

                        # m1h = (sig_g - 0.5) * i = (i*g)/2
                        m1h = steps.tile([128, 2, B_PER], f32, tag="m1h")
                        nc.vector.scalar_tensor_tensor(
                            m1h, sg[:, 0:2], 0.5, sg[:, 2:4],
                            op0=mybir.AluOpType.subtract,
                            op1=mybir.AluOpType.mult,
                        )
                        if tg == 0:
                            nc.vector.tensor_scalar_mul(c_sb, m1h, 2.0)
                        else:
                            m2 = steps.tile([128, 2, B_PER], f32, tag="m2")
                            nc.vector.tensor_mul(m2, sg[:, 4:6], c_sb)
                            # c = 2*m1h + m2
                            nc.vector.scalar_tensor_tensor(
                                c_sb, m1h, 2.0, m2,
                                op0=mybir.AluOpType.mult,
                                op1=mybir.AluOpType.add,
                            )
                        # s2c = sig(2c); h/2 = (s2c - 0.5) * o
                        s2c = steps.tile([128, 2, B_PER], f32, tag="s2c")
                        nc.scalar.activation(s2c, c_sb, SIG, scale=2.0)
                        nc.vector.scalar_tensor_tensor(
                            hist[:, tl], s2c, 0.5, sg[:, 6:8],
                            op0=mybir.AluOpType.subtract,
                            op1=mybir.AluOpType.mult,
                        )
                        # f32 output path (off the critical chain), stores h/2;
                        # the output transpose multiplies by 2*I to restore h
                        nc.vector.scalar_tensor_tensor(
                            histf_blocks[gb][:, tl], s2c, 0.5, sg[:, 6:8],
                            op0=mybir.AluOpType.subtract,
                            op1=mybir.AluOpType.mult,
                        )

                    for th in pending:
                        th()
                    pending = []

                    # block output: transpose hist back to batch-major
                    QT = TBLK * 2 * B_PER // 128  # quarters (4 for TBLK=32)
                    tq = 128 // (2 * B_PER)  # timesteps per quarter (8)
                    ob = outp.tile([128, QT, 128], f32, tag="ob")
                    for q in range(QT):
                        pt = P[:, q, 0:128]  # [128,128] f32 scratch
                        nc.tensor.transpose(
                            pt,
                            histf_blocks[gb][:, q * tq : (q + 1) * tq].rearrange(
                                "p t c b -> p (t c b)"
                            ),
                            ident,
                        )
                        nc.vector.tensor_scalar_mul(ob[:, q, :], pt, 2.0)
                        dst = hs_d[
                            :, b * TBLK + q * tq : b * TBLK + (q + 1) * tq, :
                        ].rearrange("b t (c p) -> t c b p", c=2)
                        nc.sync.dma_start(out=dst, in_=ob[:, q, :])

                # final cell state: transpose [128,(2,8)] -> [(2,8),128]
                ctp = gates_pool.tile([128, 8, BT], f32, tag="gates")
                pt = ctp[0:16, 0, 0:128]
                nc.tensor.transpose(pt, c_sb.rearrange("p c b -> p (c b)"), ident)
                cto = outp.tile([16, 128], f32, tag="cto")
                nc.vector.tensor_copy(out=cto, in_=pt)
                nc.sync.dma_start(
                    out=ct_d.rearrange("b (c p) -> c b p", c=2), in_=cto
                )

    nc.finalize()
    return nc


def _get_nc(T=T_FULL, TBLK=32):
    key = (T, TBLK)
    if key not in _NC_CACHE:
        _NC_CACHE[key] = _build_nc(T, TBLK)
    return _NC_CACHE[key]


def _host_prep(Wi, Wh, B):
    """Permute gate blocks (i,f,g,o) -> (g,i,f,o); cast for device."""
    perm = np.concatenate(
        [
            np.arange(2 * H, 3 * H),  # g
            np.arange(0, H),  # i
            np.arange(H, 2 * H),  # f
            np.arange(3 * H, 4 * H),  # o
        ]
    )
    Wi_p = np.ascontiguousarray(np.asarray(Wi, np.float32)[:, perm])
    Wh_p = np.asarray(Wh, np.float32)[:, perm]
    B_p = np.asarray(B, np.float32)[perm]
    # sigma-everywhere scaling: g-gate columns x2 (tanh(x) = 2*sig(2x)-1),
    # all Wh columns x2 again because h is stored as h/2 on device
    Wi_p[:, :H] *= 2.0
    B_p[:H] *= 2.0
    Wh_p = Wh_p * 2.0
    Wh_p[:, :H] *= 2.0
    Wh_bf = np.ascontiguousarray(Wh_p.astype(ml_dtypes.bfloat16))
    b_hi = B_p.astype(ml_dtypes.bfloat16)
    b_lo = (B_p - b_hi.astype(np.float32)).astype(ml_dtypes.bfloat16)
    return Wi_p, Wh_bf, b_hi.reshape(1, G4), b_lo.reshape(1, G4)


def kernel(x, Wi, Wh, B):
    from concourse.bass_utils import run_bass_kernel_spmd

    x = np.asarray(x, np.float32)
    Wi_p, Wh_bf, b_hi, b_lo = _host_prep(Wi, Wh, B)
    nc = _get_nc()

    in_maps = []
    for c in range(NCORES):
        in_maps.append(
            {
                "x": np.ascontiguousarray(x[c * B_PER : (c + 1) * B_PER]),
                "wi": Wi_p,
                "wh": Wh_bf,
                "bhi": b_hi,
                "blo": b_lo,
            }
        )
    res = run_bass_kernel_spmd(nc, in_maps, list(range(NCORES)))
    hs = np.concatenate([res.results[c]["out_hs"] for c in range(NCORES)], axis=0)
    ct = np.concatenate([res.results[c]["out_ct"] for c in range(NCORES)], axis=0)
    ht = np.ascontiguousarray(hs[:, -1, :])
    return (hs, (ht, ct))
